# revision 9
# baseline (speedup 1.0000x reference)
"""Trainium2 Bass kernel for nn_BaseAggregator_31439160607279.

Math (reference):
  af (a,c,f,t), imf (v,c,h,w), split c into k=2 heads of 256 ch.
  sims[a,v,k,hw,t] = sum_c af*imf ; + cls[a,v,k] ; relu ; max over hw ;
  masked mean over t (mask m[a,t] in {0,1}, den = f*sum_t m) ; sum over k.

Strategy:
  - Shard the image dim v=32 across 8 cores (4 images/core); audio replicated.
  - Pack ALL mask-active (a, t) pairs (all 32 audios) into the matmul M dim
    (m=0 columns contribute nothing to the masked sum) -> ~3219 rows -> 26
    M-tiles of 128 with ~3% padding.
  - Big matmuls in fp16 (PE upcasts to FP22; 1 cycle/row streaming):
      lhsT = packed audio rows, channel-chunk (K=128, M=128)
      rhs  = [imf[v0,k] | imf[v1,k]] (K=128, N=392), accumulate 2 chunks;
      two image pairs (all 4 local images) share one 2-bank PSUM tile.
  - relu(max_hw(x)+cls) == max_hw(relu(x+cls)): reduce_max on raw PSUM
    (one DVE op per 2-bank PSUM group), then add row-broadcast cls (one-hot
    matmul), relu on the Scalar engine.
  - Masked t-sum via matmul with one-hot audio columns (K=packed rows,
    M=32 audios), accumulated across M-tiles in a single PSUM bank;
  - divide by den, sum heads; host concatenates core outputs along v.
"""

import math
from contextlib import ExitStack

import numpy as np

import concourse.bacc as bacc
import concourse.mybir as mybir
import concourse.tile as tile
from concourse.bass_utils import run_bass_kernel_spmd

# Problem dims (hardcoded per spec)
A, V, C, F, T, H, W = 32, 32, 512, 1, 200, 14, 14
K = 2                    # heads
NCH = C // K             # 256 channels per head
KC = 2                   # channel chunks per head
KP = NCH // KC           # 128 = contraction per matmul
HW = H * W               # 196
NCORES = 8
VL = V // NCORES         # 4 local images per core
NVP = VL // 2            # 2 local image pairs
NPAIR = 2 * HW           # 392 = matmul free dim per image pair

GATHER = True            # pack only mask-active (a, t) rows
AFP_CHUNK = 7            # M-tiles per audio DMA chunk

TRACE = False
LAST_RESULTS = None

_kernel_cache = {}

f32 = mybir.dt.float32
f16 = mybir.dt.float16
X = mybir.AxisListType.X


def _build(MT: int):
    """Build + compile the per-core Bass program for MT packed-row tiles."""
    nc = bacc.Bacc("TRN2", target_bir_lowering=False, debug=False)

    afp_d = nc.dram_tensor("afp", (K, KC, KP, MT * 128), f16, kind="ExternalInput")
    imf_d = nc.dram_tensor("imf", (KP, K * KC * VL * HW), f16, kind="ExternalInput")
    # aux = [acls (K*KC*A=128) | icls (K*KC*VL=16) | maskc (MT*A)] along free dim
    aux_d = nc.dram_tensor("aux", (KP, 144 + MT * A), f16, kind="ExternalInput")
    onehot_d = nc.dram_tensor("onehot", (A, MT * 128), f16, kind="ExternalInput")
    maskf_d = nc.dram_tensor("maskf", (A, T), f32, kind="ExternalInput")
    outk_d = nc.dram_tensor("outk", (A, K * VL), f32, kind="ExternalOutput")
    outsum_d = nc.dram_tensor("outsum", (A, VL), f32, kind="ExternalOutput")

    with tile.TileContext(nc) as tc, ExitStack() as ctx:
        cst = ctx.enter_context(tc.tile_pool(name="cst", bufs=1))
        ps_big = ctx.enter_context(tc.tile_pool(name="ps_big", bufs=3, space="PSUM"))
        ps_sm = ctx.enter_context(tc.tile_pool(name="ps_sm", bufs=1, space="PSUM"))
        ps_num = ctx.enter_context(tc.tile_pool(name="ps_num", bufs=1, space="PSUM"))
        sm_pool = ctx.enter_context(tc.tile_pool(name="sm", bufs=3))

        # --- persistent SBUF tiles ---
        afp_sb = {}
        for k in range(K):
            for kc in range(KC):
                afp_sb[k, kc] = cst.tile([KP, MT * 128], f16, tag=f"afp{k}{kc}", name=f"afp{k}{kc}")
        imf_sb = cst.tile([KP, K * KC * VL * HW], f16, tag="imf", name="imf_sb")
        aux_sb = cst.tile([KP, 144 + MT * A], f16, tag="aux", name="aux_sb")
        onehot_sb = cst.tile([A, MT * 128], f16, tag="onehot", name="onehot_sb")
        maskf_sb = cst.tile([A, T], f32, tag="maskf", name="maskf_sb")

        def imf_rhs(k, kc, sub):
            off = (k * KC + kc) * (VL * HW) + sub * NPAIR
            return imf_sb[:, off:off + NPAIR]

        def acls_lhs(k, kc):
            off = (k * KC + kc) * A
            return aux_sb[:, off:off + A]

        def icls_rhs(k, kc):
            off = 128 + (k * KC + kc) * VL
            return aux_sb[:, off:off + VL]

        def maskc_lhs(mt):
            off = 144 + mt * A
            return aux_sb[:, off:off + A]

        # DMA order: compute-gating transfers first, all with fat descriptors.
        half = K * KC * VL * HW // 2
        nc.sync.dma_start(out=imf_sb[:, 0:half], in_=imf_d.ap()[:, 0:half])
        c0 = min(MT, AFP_CHUNK) * 128
        for k in range(K):
            for kc in range(KC):
                nc.sync.dma_start(out=afp_sb[k, kc][:, 0:c0], in_=afp_d.ap()[k, kc][:, 0:c0])
        nc.sync.dma_start(out=imf_sb[:, half:2 * half], in_=imf_d.ap()[:, half:2 * half])
        nc.sync.dma_start(out=aux_sb[:], in_=aux_d.ap())
        nc.sync.dma_start(out=onehot_sb[:], in_=onehot_d.ap())
        n_chunks = math.ceil(MT / AFP_CHUNK)
        for ch in range(1, n_chunks):
            sl = slice(ch * AFP_CHUNK * 128, min(MT, (ch + 1) * AFP_CHUNK) * 128)
            for k in range(K):
                for kc in range(KC):
                    nc.sync.dma_start(out=afp_sb[k, kc][:, sl], in_=afp_d.ap()[k, kc][:, sl])
        nc.sync.dma_start(out=maskf_sb[:], in_=maskf_d.ap())

        # --- PE warm-up: keep the PE busy during the input DMA so the HAM
        # clock-gate reaches 8/8 before the real matmuls arrive ---
        warm = cst.tile([KP, 512], f16, tag="warm", name="warm_sb")
        nc.vector.memset(warm[:], 0.0)
        for w in range(14):
            pw = ps_big.tile([128, 1024], f32, tag="ps_big", name="ps_warm")
            nc.tensor.matmul(pw[:, 0:512], lhsT=warm[:, 0:128], rhs=warm[:],
                             start=True, stop=True)

        # --- main loop over M-tiles (cls_sims emitted after mt0's sims MMs) ---
        cls_sb = cst.tile([A, K * VL], f16, tag="cls", name="cls_sb")
        clsb_sb = cst.tile([128, MT * K * VL], f16, tag="clsb", name="clsb_sb")
        num_ps = ps_num.tile([A, K * VL], f32, tag="ps_num", name="ps_numacc")
        for mt in range(MT):
            smraw = sm_pool.tile([128, K * VL], f16, tag="smraw", name="smraw")
            for k in range(K):
                ps = ps_big.tile([128, 1024], f32, tag="ps_big", name="ps_sims")
                for sub in range(NVP):
                    for kc in range(KC):
                        nc.tensor.matmul(
                            ps[:, sub * 512:sub * 512 + NPAIR],
                            lhsT=afp_sb[k, kc][:, mt * 128:(mt + 1) * 128],
                            rhs=imf_rhs(k, kc, sub),
                            start=(kc == 0), stop=(kc == 1),
                        )
                rview = ps[:].rearrange("p (b q) -> p b q", b=2)[:, :, 0:NPAIR]
                rview = rview.rearrange("p b (i x) -> p b i x", i=2)
                nc.vector.reduce_max(smraw[:, k * VL:(k + 1) * VL], rview, axis=X)

            if mt == 0:
                for k in range(K):
                    pc = ps_sm.tile([A, VL], f32, tag="ps_sm", name="ps_cls")
                    for kc in range(KC):
                        nc.tensor.matmul(
                            pc[:], lhsT=acls_lhs(k, kc), rhs=icls_rhs(k, kc),
                            start=(kc == 0), stop=(kc == 1),
                        )
                    nc.vector.tensor_copy(cls_sb[:, k * VL:(k + 1) * VL], pc[:])
                # broadcast cls to every packed row once, for all M-tiles
                for bt in range(MT):
                    pb = ps_sm.tile([128, K * VL], f32, tag="ps_sm", name="ps_bcast")
                    nc.tensor.matmul(pb[:], lhsT=onehot_sb[:, bt * 128:(bt + 1) * 128],
                                     rhs=cls_sb[:], start=True, stop=True)
                    nc.scalar.copy(clsb_sb[:, bt * K * VL:(bt + 1) * K * VL], pb[:])

            sm2 = sm_pool.tile([128, K * VL], f16, tag="sm2", name="sm2")
            nc.vector.tensor_add(sm2[:], smraw[:],
                                 clsb_sb[:, mt * K * VL:(mt + 1) * K * VL])
            sm3 = sm_pool.tile([128, K * VL], f16, tag="sm3", name="sm3")
            nc.scalar.activation(sm3[:], sm2[:], mybir.ActivationFunctionType.Relu)
            nc.tensor.matmul(num_ps[:], lhsT=maskc_lhs(mt), rhs=sm3[:],
                             start=(mt == 0), stop=(mt == MT - 1))

        # --- den, divide, head-sum, out ---
        den = cst.tile([A, 1], f32, tag="den", name="den")
        nc.vector.reduce_sum(den[:], maskf_sb[:], axis=X)
        rden = cst.tile([A, 1], f32, tag="rden", name="rden")
        nc.vector.reciprocal(rden[:], den[:])
        outk_sb = cst.tile([A, K * VL], f32, tag="outk", name="outk_sb")
        nc.vector.tensor_scalar_mul(outk_sb[:], num_ps[:], rden[:])
        outsum_sb = cst.tile([A, VL], f32, tag="outsum", name="outsum_sb")
        nc.vector.tensor_add(outsum_sb[:], outk_sb[:, 0:VL], outk_sb[:, VL:2 * VL])
        nc.sync.dma_start(out=outk_d.ap(), in_=outk_sb[:])
        nc.sync.dma_start(out=outsum_d.ap(), in_=outsum_sb[:])

    nc.compile()
    return nc


def prepare_inputs(audio_feats, image_feats, audio_cls, image_cls, audio_mask):
    """Host-side shard + layout prep. Returns (MT, in_maps)."""
    af = np.ascontiguousarray(audio_feats, dtype=np.float32).reshape(A, K, KC, KP, T)
    imf = np.ascontiguousarray(image_feats, dtype=np.float32).reshape(V, K, KC, KP, HW)
    acls = np.ascontiguousarray(audio_cls, dtype=np.float32).reshape(A, K, KC, KP)
    icls = np.ascontiguousarray(image_cls, dtype=np.float32).reshape(V, K, KC, KP)
    mask = np.asarray(audio_mask)
    maskf = np.ascontiguousarray(mask.astype(np.float32))

    if GATHER:
        rows_a, rows_t = np.nonzero(mask != 0)
        mvals = np.ones(len(rows_a), np.float32)
    else:
        rows_a, rows_t = np.indices((A, T)).reshape(2, -1)
        mvals = maskf[rows_a, rows_t]
    L = len(rows_a)
    MT = max(1, math.ceil(L / 128))
    LP = MT * 128

    # audio rows, shared by all cores: (K, KC, KP, MT*128) fp16
    af_rows = np.zeros((LP, K, KC, KP), np.float32)
    af_rows[:L] = af[rows_a, :, :, :, rows_t]
    afp = np.ascontiguousarray(
        af_rows.transpose(1, 2, 3, 0).reshape(K, KC, KP, MT * 128)
    ).astype(np.float16)

    oh = np.zeros((LP, A), np.float16)
    oh[np.arange(L), rows_a] = 1.0
    onehot = np.ascontiguousarray(oh.T)                       # (A, MT*128)
    mc = np.zeros((LP, A), np.float16)
    mc[np.arange(L), rows_a] = mvals
    maskc = mc.reshape(MT, 128, A).transpose(1, 0, 2).reshape(128, MT * A)
    acls_h = acls.transpose(3, 1, 2, 0).reshape(KP, K * KC * A)

    in_maps = []
    for ci in range(NCORES):
        vsl = slice(ci * VL, (ci + 1) * VL)
        imf_h = np.ascontiguousarray(
            imf[vsl].transpose(3, 1, 2, 0, 4).reshape(KP, K * KC * VL * HW)
        ).astype(np.float16)
        icls_h = icls[vsl].transpose(3, 1, 2, 0).reshape(KP, K * KC * VL)
        aux = np.concatenate(
            [acls_h, icls_h, maskc], axis=1
        ).astype(np.float16)
        in_maps.append({
            "afp": afp,
            "imf": imf_h,
            "aux": np.ascontiguousarray(aux),
            "onehot": onehot,
            "maskf": maskf,
        })
    return MT, in_maps


def get_program(MT: int):
    if MT not in _kernel_cache:
        _kernel_cache[MT] = _build(MT)
    return _kernel_cache[MT]


def kernel(audio_feats, image_feats, audio_cls, image_cls, audio_mask, agg_heads):
    global LAST_RESULTS
    MT, in_maps = prepare_inputs(
        audio_feats, image_feats, audio_cls, image_cls, audio_mask
    )
    nc = get_program(MT)
    res = run_bass_kernel_spmd(nc, in_maps, list(range(NCORES)), trace=TRACE)
    LAST_RESULTS = res
    agg = bool(np.asarray(agg_heads))
    outs = []
    for ci in range(NCORES):
        if agg:
            outs.append(res.results[ci]["outsum"])  # (A, VL)
        else:
            outk = res.results[ci]["outk"].reshape(A, K, VL)
            outs.append(outk.transpose(0, 2, 1))    # (A, VL, K)
    return np.concatenate(outs, axis=1).astype(np.float32)


# revision 10
# speedup vs baseline: 1.0143x; 1.0143x over previous
"""Trainium2 Bass kernel for nn_BaseAggregator_31439160607279.

Math (reference):
  af (a,c,f,t), imf (v,c,h,w), split c into k=2 heads of 256 ch.
  sims[a,v,k,hw,t] = sum_c af*imf ; + cls[a,v,k] ; relu ; max over hw ;
  masked mean over t (mask m[a,t] in {0,1}, den = f*sum_t m) ; sum over k.

Strategy:
  - Shard the image dim v=32 across 8 cores (4 images/core); audio replicated.
  - Pack ALL mask-active (a, t) pairs (all 32 audios) into the matmul M dim
    (m=0 columns contribute nothing to the masked sum) -> ~3219 rows -> 26
    M-tiles of 128 with ~3% padding.
  - Big matmuls in fp16 (PE upcasts to FP22; 1 cycle/row streaming):
      lhsT = packed audio rows, channel-chunk (K=128, M=128)
      rhs  = [imf[v0,k] | imf[v1,k]] (K=128, N=392), accumulate 2 chunks;
      two image pairs (all 4 local images) share one 2-bank PSUM tile.
  - relu(max_hw(x)+cls) == max_hw(relu(x+cls)): reduce_max on raw PSUM
    (one DVE op per 2-bank PSUM group), then add row-broadcast cls (one-hot
    matmul), relu on the Scalar engine.
  - Masked t-sum via matmul with one-hot audio columns (K=packed rows,
    M=32 audios), accumulated across M-tiles in a single PSUM bank;
  - divide by den, sum heads; host concatenates core outputs along v.
"""

import math
from contextlib import ExitStack

import numpy as np

import concourse.bacc as bacc
import concourse.mybir as mybir
import concourse.tile as tile
from concourse.bass_utils import run_bass_kernel_spmd

# Problem dims (hardcoded per spec)
A, V, C, F, T, H, W = 32, 32, 512, 1, 200, 14, 14
K = 2                    # heads
NCH = C // K             # 256 channels per head
KC = 2                   # channel chunks per head
KP = NCH // KC           # 128 = contraction per matmul
HW = H * W               # 196
NCORES = 8
VL = V // NCORES         # 4 local images per core
NVP = VL // 2            # 2 local image pairs
NPAIR = 2 * HW           # 392 = matmul free dim per image pair

GATHER = True            # pack only mask-active (a, t) rows
AFP_CHUNK = 7            # M-tiles per audio DMA chunk

TRACE = False
LAST_RESULTS = None

_kernel_cache = {}

f32 = mybir.dt.float32
f16 = mybir.dt.float16
X = mybir.AxisListType.X


def _build(MT: int):
    """Build + compile the per-core Bass program for MT packed-row tiles."""
    nc = bacc.Bacc("TRN2", target_bir_lowering=False, debug=False)

    afp_d = nc.dram_tensor("afp", (K, KC, KP, MT * 128), f16, kind="ExternalInput")
    imf_d = nc.dram_tensor("imf", (KP, K * KC * VL * HW), f16, kind="ExternalInput")
    # aux = [acls (K*KC*A=128) | icls (K*KC*VL=16) | maskc (MT*A)] along free dim
    aux_d = nc.dram_tensor("aux", (KP, 144 + MT * A), f16, kind="ExternalInput")
    onehot_d = nc.dram_tensor("onehot", (A, MT * 128), f16, kind="ExternalInput")
    maskf_d = nc.dram_tensor("maskf", (A, T), f32, kind="ExternalInput")
    outk_d = nc.dram_tensor("outk", (A, K * VL), f32, kind="ExternalOutput")
    outsum_d = nc.dram_tensor("outsum", (A, VL), f32, kind="ExternalOutput")

    with tile.TileContext(nc) as tc, ExitStack() as ctx:
        cst = ctx.enter_context(tc.tile_pool(name="cst", bufs=1))
        ps_big = ctx.enter_context(tc.tile_pool(name="ps_big", bufs=3, space="PSUM"))
        ps_sm = ctx.enter_context(tc.tile_pool(name="ps_sm", bufs=1, space="PSUM"))
        ps_num = ctx.enter_context(tc.tile_pool(name="ps_num", bufs=1, space="PSUM"))
        sm_pool = ctx.enter_context(tc.tile_pool(name="sm", bufs=3))

        # --- persistent SBUF tiles ---
        afp_sb = {}
        for k in range(K):
            for kc in range(KC):
                afp_sb[k, kc] = cst.tile([KP, MT * 128], f16, tag=f"afp{k}{kc}", name=f"afp{k}{kc}")
        imf_sb = cst.tile([KP, K * KC * VL * HW], f16, tag="imf", name="imf_sb")
        aux_sb = cst.tile([KP, 144 + MT * A], f16, tag="aux", name="aux_sb")
        onehot_sb = cst.tile([A, MT * 128], f16, tag="onehot", name="onehot_sb")
        maskf_sb = cst.tile([A, T], f32, tag="maskf", name="maskf_sb")

        def imf_rhs(k, kc, sub):
            off = (k * KC + kc) * (VL * HW) + sub * NPAIR
            return imf_sb[:, off:off + NPAIR]

        def acls_lhs(k, kc):
            off = (k * KC + kc) * A
            return aux_sb[:, off:off + A]

        def icls_rhs(k, kc):
            off = 128 + (k * KC + kc) * VL
            return aux_sb[:, off:off + VL]

        def maskc_lhs(mt):
            off = 144 + mt * A
            return aux_sb[:, off:off + A]

        # DMA order: compute-gating transfers first, all with fat descriptors.
        half = K * KC * VL * HW // 2
        nc.sync.dma_start(out=imf_sb[:, 0:half], in_=imf_d.ap()[:, 0:half])
        afp_cuts = [0, min(4, MT)] + [min(MT, c) for c in range(AFP_CHUNK, MT + AFP_CHUNK, AFP_CHUNK)]
        afp_cuts = sorted(set(afp_cuts))
        c0 = afp_cuts[1] * 128
        for k in range(K):
            for kc in range(KC):
                nc.sync.dma_start(out=afp_sb[k, kc][:, 0:c0], in_=afp_d.ap()[k, kc][:, 0:c0])
        nc.sync.dma_start(out=imf_sb[:, half:2 * half], in_=imf_d.ap()[:, half:2 * half])
        nc.sync.dma_start(out=aux_sb[:], in_=aux_d.ap())
        nc.sync.dma_start(out=onehot_sb[:], in_=onehot_d.ap())
        for lo, hi in zip(afp_cuts[1:-1], afp_cuts[2:]):
            sl = slice(lo * 128, hi * 128)
            for k in range(K):
                for kc in range(KC):
                    nc.sync.dma_start(out=afp_sb[k, kc][:, sl], in_=afp_d.ap()[k, kc][:, sl])
        nc.sync.dma_start(out=maskf_sb[:], in_=maskf_d.ap())

        # --- PE warm-up: keep the PE busy during the input DMA so the HAM
        # clock-gate reaches 8/8 before the real matmuls arrive ---
        warm = cst.tile([KP, 512], f16, tag="warm", name="warm_sb")
        nc.vector.memset(warm[:], 0.0)
        for w in range(8):
            pw = ps_big.tile([128, 1024], f32, tag="ps_big", name="ps_warm")
            nc.tensor.matmul(pw[:, 0:512], lhsT=warm[:, 0:128], rhs=warm[:],
                             start=True, stop=True)

        # --- main loop over M-tiles (cls_sims emitted after mt0's sims MMs) ---
        cls_sb = cst.tile([A, K * VL], f16, tag="cls", name="cls_sb")
        clsb_sb = cst.tile([128, MT * K * VL], f16, tag="clsb", name="clsb_sb")
        num_ps = ps_num.tile([A, K * VL], f32, tag="ps_num", name="ps_numacc")
        for mt in range(MT):
            smraw = sm_pool.tile([128, K * VL], f16, tag="smraw", name="smraw")
            for k in range(K):
                ps = ps_big.tile([128, 1024], f32, tag="ps_big", name="ps_sims")
                for sub in range(NVP):
                    for kc in range(KC):
                        nc.tensor.matmul(
                            ps[:, sub * 512:sub * 512 + NPAIR],
                            lhsT=afp_sb[k, kc][:, mt * 128:(mt + 1) * 128],
                            rhs=imf_rhs(k, kc, sub),
                            start=(kc == 0), stop=(kc == 1),
                        )
                rview = ps[:].rearrange("p (b q) -> p b q", b=2)[:, :, 0:NPAIR]
                rview = rview.rearrange("p b (i x) -> p b i x", i=2)
                nc.vector.reduce_max(smraw[:, k * VL:(k + 1) * VL], rview, axis=X)

            if mt == 0:
                for k in range(K):
                    pc = ps_sm.tile([A, VL], f32, tag="ps_sm", name="ps_cls")
                    for kc in range(KC):
                        nc.tensor.matmul(
                            pc[:], lhsT=acls_lhs(k, kc), rhs=icls_rhs(k, kc),
                            start=(kc == 0), stop=(kc == 1),
                        )
                    nc.vector.tensor_copy(cls_sb[:, k * VL:(k + 1) * VL], pc[:])
                # broadcast cls to every packed row once, for all M-tiles
                for bt in range(MT):
                    pb = ps_sm.tile([128, K * VL], f32, tag="ps_sm", name="ps_bcast")
                    nc.tensor.matmul(pb[:], lhsT=onehot_sb[:, bt * 128:(bt + 1) * 128],
                                     rhs=cls_sb[:], start=True, stop=True)
                    nc.scalar.copy(clsb_sb[:, bt * K * VL:(bt + 1) * K * VL], pb[:])

            sm2 = sm_pool.tile([128, K * VL], f16, tag="sm2", name="sm2")
            nc.vector.tensor_add(sm2[:], smraw[:],
                                 clsb_sb[:, mt * K * VL:(mt + 1) * K * VL])
            sm3 = sm_pool.tile([128, K * VL], f16, tag="sm3", name="sm3")
            nc.scalar.activation(sm3[:], sm2[:], mybir.ActivationFunctionType.Relu)
            nc.tensor.matmul(num_ps[:], lhsT=maskc_lhs(mt), rhs=sm3[:],
                             start=(mt == 0), stop=(mt == MT - 1))

        # --- den, divide, head-sum, out ---
        den = cst.tile([A, 1], f32, tag="den", name="den")
        nc.vector.reduce_sum(den[:], maskf_sb[:], axis=X)
        rden = cst.tile([A, 1], f32, tag="rden", name="rden")
        nc.vector.reciprocal(rden[:], den[:])
        outk_sb = cst.tile([A, K * VL], f32, tag="outk", name="outk_sb")
        nc.vector.tensor_scalar_mul(outk_sb[:], num_ps[:], rden[:])
        outsum_sb = cst.tile([A, VL], f32, tag="outsum", name="outsum_sb")
        nc.vector.tensor_add(outsum_sb[:], outk_sb[:, 0:VL], outk_sb[:, VL:2 * VL])
        nc.sync.dma_start(out=outk_d.ap(), in_=outk_sb[:])
        nc.sync.dma_start(out=outsum_d.ap(), in_=outsum_sb[:])

    nc.compile()
    return nc


def prepare_inputs(audio_feats, image_feats, audio_cls, image_cls, audio_mask):
    """Host-side shard + layout prep. Returns (MT, in_maps)."""
    af = np.ascontiguousarray(audio_feats, dtype=np.float32).reshape(A, K, KC, KP, T)
    imf = np.ascontiguousarray(image_feats, dtype=np.float32).reshape(V, K, KC, KP, HW)
    acls = np.ascontiguousarray(audio_cls, dtype=np.float32).reshape(A, K, KC, KP)
    icls = np.ascontiguousarray(image_cls, dtype=np.float32).reshape(V, K, KC, KP)
    mask = np.asarray(audio_mask)
    maskf = np.ascontiguousarray(mask.astype(np.float32))

    if GATHER:
        rows_a, rows_t = np.nonzero(mask != 0)
        mvals = np.ones(len(rows_a), np.float32)
    else:
        rows_a, rows_t = np.indices((A, T)).reshape(2, -1)
        mvals = maskf[rows_a, rows_t]
    L = len(rows_a)
    MT = max(1, math.ceil(L / 128))
    LP = MT * 128

    # audio rows, shared by all cores: (K, KC, KP, MT*128) fp16
    af_rows = np.zeros((LP, K, KC, KP), np.float32)
    af_rows[:L] = af[rows_a, :, :, :, rows_t]
    afp = np.ascontiguousarray(
        af_rows.transpose(1, 2, 3, 0).reshape(K, KC, KP, MT * 128)
    ).astype(np.float16)

    oh = np.zeros((LP, A), np.float16)
    oh[np.arange(L), rows_a] = 1.0
    onehot = np.ascontiguousarray(oh.T)                       # (A, MT*128)
    mc = np.zeros((LP, A), np.float16)
    mc[np.arange(L), rows_a] = mvals
    maskc = mc.reshape(MT, 128, A).transpose(1, 0, 2).reshape(128, MT * A)
    acls_h = acls.transpose(3, 1, 2, 0).reshape(KP, K * KC * A)

    in_maps = []
    for ci in range(NCORES):
        vsl = slice(ci * VL, (ci + 1) * VL)
        imf_h = np.ascontiguousarray(
            imf[vsl].transpose(3, 1, 2, 0, 4).reshape(KP, K * KC * VL * HW)
        ).astype(np.float16)
        icls_h = icls[vsl].transpose(3, 1, 2, 0).reshape(KP, K * KC * VL)
        aux = np.concatenate(
            [acls_h, icls_h, maskc], axis=1
        ).astype(np.float16)
        in_maps.append({
            "afp": afp,
            "imf": imf_h,
            "aux": np.ascontiguousarray(aux),
            "onehot": onehot,
            "maskf": maskf,
        })
    return MT, in_maps


def get_program(MT: int):
    if MT not in _kernel_cache:
        _kernel_cache[MT] = _build(MT)
    return _kernel_cache[MT]


def kernel(audio_feats, image_feats, audio_cls, image_cls, audio_mask, agg_heads):
    global LAST_RESULTS
    MT, in_maps = prepare_inputs(
        audio_feats, image_feats, audio_cls, image_cls, audio_mask
    )
    nc = get_program(MT)
    res = run_bass_kernel_spmd(nc, in_maps, list(range(NCORES)), trace=TRACE)
    LAST_RESULTS = res
    agg = bool(np.asarray(agg_heads))
    outs = []
    for ci in range(NCORES):
        if agg:
            outs.append(res.results[ci]["outsum"])  # (A, VL)
        else:
            outk = res.results[ci]["outk"].reshape(A, K, VL)
            outs.append(outk.transpose(0, 2, 1))    # (A, VL, K)
    return np.concatenate(outs, axis=1).astype(np.float32)


# revision 12
# speedup vs baseline: 1.0734x; 1.0583x over previous
"""Trainium2 Bass kernel for nn_BaseAggregator_31439160607279.

Math (reference):
  af (a,c,f,t), imf (v,c,h,w), split c into k=2 heads of 256 ch.
  sims[a,v,k,hw,t] = sum_c af*imf ; + cls[a,v,k] ; relu ; max over hw ;
  masked mean over t (mask m[a,t] in {0,1}, den = f*sum_t m) ; sum over k.

Strategy:
  - Shard the image dim v=32 across 8 cores (4 images/core); audio replicated.
  - Pack ALL mask-active (a, t) pairs (all 32 audios) into the matmul M dim
    (m=0 columns contribute nothing to the masked sum) -> ~3219 rows -> 26
    M-tiles of 128 with ~3% padding.
  - Big matmuls in fp16 (PE upcasts to FP22; 1 cycle/row streaming):
      lhsT = packed audio rows, channel-chunk (K=128, M=128)
      rhs  = [imf[v0,k] | imf[v1,k]] (K=128, N=392), accumulate 2 chunks;
      two image pairs (all 4 local images) share one 2-bank PSUM tile.
  - relu(max_hw(x)+cls) == max_hw(relu(x+cls)): reduce_max on raw PSUM
    (one DVE op per 2-bank PSUM group), then add row-broadcast cls (one-hot
    matmul), relu on the Scalar engine.
  - Masked t-sum via matmul with one-hot audio columns (K=packed rows,
    M=32 audios), accumulated across M-tiles in a single PSUM bank;
  - divide by den, sum heads; host concatenates core outputs along v.
"""

import math
from contextlib import ExitStack

import numpy as np

import concourse.bacc as bacc
import concourse.mybir as mybir
import concourse.tile as tile
from concourse.bass_utils import run_bass_kernel_spmd

# Problem dims (hardcoded per spec)
A, V, C, F, T, H, W = 32, 32, 512, 1, 200, 14, 14
K = 2                    # heads
NCH = C // K             # 256 channels per head
KC = 2                   # channel chunks per head
KP = NCH // KC           # 128 = contraction per matmul
HW = H * W               # 196
NCORES = 8
VL = V // NCORES         # 4 local images per core
NVP = VL // 2            # 2 local image pairs
NPAIR = 2 * HW           # 392 = matmul free dim per image pair

GATHER = True            # pack only mask-active (a, t) rows
AFP_CHUNK = 7            # M-tiles per audio DMA chunk

TRACE = False
LAST_RESULTS = None

_kernel_cache = {}

f32 = mybir.dt.float32
f16 = mybir.dt.float16
X = mybir.AxisListType.X


def _build(MT: int):
    """Build + compile the per-core Bass program for MT packed-row tiles."""
    nc = bacc.Bacc("TRN2", target_bir_lowering=False, debug=False)

    afp_d = nc.dram_tensor("afp", (K, KC, KP, MT * 128), f16, kind="ExternalInput")
    imf_d = nc.dram_tensor("imf", (KP, K * KC * VL * HW), f16, kind="ExternalInput")
    # aux = [acls (K*KC*A=128) | icls (K*KC*VL=16) | maskc (MT*A)] along free dim
    aux_d = nc.dram_tensor("aux", (KP, 144 + MT * A), f16, kind="ExternalInput")
    onehot_d = nc.dram_tensor("onehot", (A, MT * 128), f16, kind="ExternalInput")
    maskf_d = nc.dram_tensor("maskf", (A, T), f32, kind="ExternalInput")
    outk_d = nc.dram_tensor("outk", (A, K * VL), f32, kind="ExternalOutput")
    outsum_d = nc.dram_tensor("outsum", (A, VL), f32, kind="ExternalOutput")

    with tile.TileContext(nc) as tc, ExitStack() as ctx:
        cst = ctx.enter_context(tc.tile_pool(name="cst", bufs=1))
        ps_big = ctx.enter_context(tc.tile_pool(name="ps_big", bufs=3, space="PSUM"))
        ps_sm = ctx.enter_context(tc.tile_pool(name="ps_sm", bufs=1, space="PSUM"))
        ps_num = ctx.enter_context(tc.tile_pool(name="ps_num", bufs=1, space="PSUM"))
        sm_pool = ctx.enter_context(tc.tile_pool(name="sm", bufs=3))

        # --- persistent SBUF tiles ---
        afp_sb = {}
        for k in range(K):
            for kc in range(KC):
                afp_sb[k, kc] = cst.tile([KP, MT * 128], f16, tag=f"afp{k}{kc}", name=f"afp{k}{kc}")
        imf_sb = cst.tile([KP, K * KC * VL * HW], f16, tag="imf", name="imf_sb")
        aux_sb = cst.tile([KP, 144 + MT * A], f16, tag="aux", name="aux_sb")
        onehot_sb = cst.tile([A, MT * 128], f16, tag="onehot", name="onehot_sb")
        maskf_sb = cst.tile([A, T], f32, tag="maskf", name="maskf_sb")

        def imf_rhs(k, kc, sub):
            off = (k * KC + kc) * (VL * HW) + sub * NPAIR
            return imf_sb[:, off:off + NPAIR]

        def acls_lhs(k, kc):
            off = (k * KC + kc) * A
            return aux_sb[:, off:off + A]

        def icls_rhs(k, kc):
            off = 128 + (k * KC + kc) * VL
            return aux_sb[:, off:off + VL]

        def maskc_lhs(mt):
            off = 144 + mt * A
            return aux_sb[:, off:off + A]

        # DMA order: compute-gating transfers first, all with fat descriptors.
        half = K * KC * VL * HW // 2
        nc.sync.dma_start(out=imf_sb[:, 0:half], in_=imf_d.ap()[:, 0:half])
        afp_cuts = [0, min(4, MT)] + [min(MT, c) for c in range(AFP_CHUNK, MT + AFP_CHUNK, AFP_CHUNK)]
        afp_cuts = sorted(set(afp_cuts))
        c0 = afp_cuts[1] * 128
        for k in range(K):
            for kc in range(KC):
                nc.sync.dma_start(out=afp_sb[k, kc][:, 0:c0], in_=afp_d.ap()[k, kc][:, 0:c0])
        nc.sync.dma_start(out=imf_sb[:, half:2 * half], in_=imf_d.ap()[:, half:2 * half])
        nc.sync.dma_start(out=aux_sb[:], in_=aux_d.ap())
        nc.sync.dma_start(out=onehot_sb[:], in_=onehot_d.ap())
        for lo, hi in zip(afp_cuts[1:-1], afp_cuts[2:]):
            sl = slice(lo * 128, hi * 128)
            for k in range(K):
                for kc in range(KC):
                    nc.sync.dma_start(out=afp_sb[k, kc][:, sl], in_=afp_d.ap()[k, kc][:, sl])
        nc.sync.dma_start(out=maskf_sb[:], in_=maskf_d.ap())

        # --- PE warm-up: keep the PE busy during the input DMA so the HAM
        # clock-gate reaches 8/8 before the real matmuls arrive ---
        warm = cst.tile([KP, 512], f16, tag="warm", name="warm_sb")
        nc.vector.memset(warm[:], 0.0)
        for w in range(8):
            pw = ps_big.tile([128, 1024], f32, tag="ps_big", name="ps_warm")
            nc.tensor.matmul(pw[:, 0:512], lhsT=warm[:, 0:128], rhs=warm[:],
                             start=True, stop=True)

        # --- main loop over M-tiles (cls_sims emitted after mt0's sims MMs) ---
        cls_sb = cst.tile([A, K * VL], f16, tag="cls", name="cls_sb")
        clsb_sb = cst.tile([128, MT * K * VL], f16, tag="clsb", name="clsb_sb")
        num_ps = ps_num.tile([A, K * VL], f32, tag="ps_num", name="ps_numacc")
        sm3_tiles = []
        for mt in range(MT):
            smraw = sm_pool.tile([128, K * VL], f16, tag="smraw", name="smraw")
            for k in range(K):
                ps = ps_big.tile([128, 1024], f32, tag="ps_big", name="ps_sims")
                for sub in range(NVP):
                    for kc in range(KC):
                        nc.tensor.matmul(
                            ps[:, sub * 512:sub * 512 + NPAIR],
                            lhsT=afp_sb[k, kc][:, mt * 128:(mt + 1) * 128],
                            rhs=imf_rhs(k, kc, sub),
                            start=(kc == 0), stop=(kc == 1),
                        )
                rview = ps[:].rearrange("p (b q) -> p b q", b=2)[:, :, 0:NPAIR]
                rview = rview.rearrange("p b (i x) -> p b i x", i=2)
                nc.vector.reduce_max(smraw[:, k * VL:(k + 1) * VL], rview, axis=X)

            if mt == 0:
                for k in range(K):
                    pc = ps_sm.tile([A, VL], f32, tag="ps_sm", name="ps_cls")
                    for kc in range(KC):
                        nc.tensor.matmul(
                            pc[:], lhsT=acls_lhs(k, kc), rhs=icls_rhs(k, kc),
                            start=(kc == 0), stop=(kc == 1),
                        )
                    nc.vector.tensor_copy(cls_sb[:, k * VL:(k + 1) * VL], pc[:])
            # spread the cls row-broadcasts over the first iterations
            for bt in range(2 * mt, min(2 * mt + 2, MT)):
                pb = ps_sm.tile([128, K * VL], f32, tag="ps_sm", name="ps_bcast")
                nc.tensor.matmul(pb[:], lhsT=onehot_sb[:, bt * 128:(bt + 1) * 128],
                                 rhs=cls_sb[:], start=True, stop=True)
                nc.scalar.copy(clsb_sb[:, bt * K * VL:(bt + 1) * K * VL], pb[:])

            sm2 = sm_pool.tile([128, K * VL], f16, tag="sm2", name="sm2")
            nc.gpsimd.tensor_add(sm2[:], smraw[:],
                                 clsb_sb[:, mt * K * VL:(mt + 1) * K * VL])
            sm3 = sm_pool.tile([128, K * VL], f16, tag="sm3", name="sm3", bufs=6)
            nc.scalar.activation(sm3[:], sm2[:], mybir.ActivationFunctionType.Relu)
            sm3_tiles.append(sm3)
            # emit the masked-sum matmul 2 iterations late so its relu
            # dependency never stalls the PE queue
            if mt >= 2:
                j = mt - 2
                nc.tensor.matmul(num_ps[:], lhsT=maskc_lhs(j), rhs=sm3_tiles[j][:],
                                 start=(j == 0), stop=(j == MT - 1))
        for j in range(max(MT - 2, 0), MT):
            nc.tensor.matmul(num_ps[:], lhsT=maskc_lhs(j), rhs=sm3_tiles[j][:],
                             start=(j == 0), stop=(j == MT - 1))

        # --- den, divide, head-sum, out ---
        den = cst.tile([A, 1], f32, tag="den", name="den")
        nc.vector.reduce_sum(den[:], maskf_sb[:], axis=X)
        rden = cst.tile([A, 1], f32, tag="rden", name="rden")
        nc.vector.reciprocal(rden[:], den[:])
        outk_sb = cst.tile([A, K * VL], f32, tag="outk", name="outk_sb")
        nc.vector.tensor_scalar_mul(outk_sb[:], num_ps[:], rden[:])
        outsum_sb = cst.tile([A, VL], f32, tag="outsum", name="outsum_sb")
        nc.vector.tensor_add(outsum_sb[:], outk_sb[:, 0:VL], outk_sb[:, VL:2 * VL])
        nc.sync.dma_start(out=outk_d.ap(), in_=outk_sb[:])
        nc.sync.dma_start(out=outsum_d.ap(), in_=outsum_sb[:])

    nc.compile()
    return nc


def prepare_inputs(audio_feats, image_feats, audio_cls, image_cls, audio_mask):
    """Host-side shard + layout prep. Returns (MT, in_maps)."""
    af = np.ascontiguousarray(audio_feats, dtype=np.float32).reshape(A, K, KC, KP, T)
    imf = np.ascontiguousarray(image_feats, dtype=np.float32).reshape(V, K, KC, KP, HW)
    acls = np.ascontiguousarray(audio_cls, dtype=np.float32).reshape(A, K, KC, KP)
    icls = np.ascontiguousarray(image_cls, dtype=np.float32).reshape(V, K, KC, KP)
    mask = np.asarray(audio_mask)
    maskf = np.ascontiguousarray(mask.astype(np.float32))

    if GATHER:
        rows_a, rows_t = np.nonzero(mask != 0)
        mvals = np.ones(len(rows_a), np.float32)
    else:
        rows_a, rows_t = np.indices((A, T)).reshape(2, -1)
        mvals = maskf[rows_a, rows_t]
    L = len(rows_a)
    MT = max(1, math.ceil(L / 128))
    LP = MT * 128

    # audio rows, shared by all cores: (K, KC, KP, MT*128) fp16
    af_rows = np.zeros((LP, K, KC, KP), np.float32)
    af_rows[:L] = af[rows_a, :, :, :, rows_t]
    afp = np.ascontiguousarray(
        af_rows.transpose(1, 2, 3, 0).reshape(K, KC, KP, MT * 128)
    ).astype(np.float16)

    oh = np.zeros((LP, A), np.float16)
    oh[np.arange(L), rows_a] = 1.0
    onehot = np.ascontiguousarray(oh.T)                       # (A, MT*128)
    mc = np.zeros((LP, A), np.float16)
    mc[np.arange(L), rows_a] = mvals
    maskc = mc.reshape(MT, 128, A).transpose(1, 0, 2).reshape(128, MT * A)
    acls_h = acls.transpose(3, 1, 2, 0).reshape(KP, K * KC * A)

    in_maps = []
    for ci in range(NCORES):
        vsl = slice(ci * VL, (ci + 1) * VL)
        imf_h = np.ascontiguousarray(
            imf[vsl].transpose(3, 1, 2, 0, 4).reshape(KP, K * KC * VL * HW)
        ).astype(np.float16)
        icls_h = icls[vsl].transpose(3, 1, 2, 0).reshape(KP, K * KC * VL)
        aux = np.concatenate(
            [acls_h, icls_h, maskc], axis=1
        ).astype(np.float16)
        in_maps.append({
            "afp": afp,
            "imf": imf_h,
            "aux": np.ascontiguousarray(aux),
            "onehot": onehot,
            "maskf": maskf,
        })
    return MT, in_maps


def get_program(MT: int):
    if MT not in _kernel_cache:
        _kernel_cache[MT] = _build(MT)
    return _kernel_cache[MT]


def kernel(audio_feats, image_feats, audio_cls, image_cls, audio_mask, agg_heads):
    global LAST_RESULTS
    MT, in_maps = prepare_inputs(
        audio_feats, image_feats, audio_cls, image_cls, audio_mask
    )
    nc = get_program(MT)
    res = run_bass_kernel_spmd(nc, in_maps, list(range(NCORES)), trace=TRACE)
    LAST_RESULTS = res
    agg = bool(np.asarray(agg_heads))
    outs = []
    for ci in range(NCORES):
        if agg:
            outs.append(res.results[ci]["outsum"])  # (A, VL)
        else:
            outk = res.results[ci]["outk"].reshape(A, K, VL)
            outs.append(outk.transpose(0, 2, 1))    # (A, VL, K)
    return np.concatenate(outs, axis=1).astype(np.float32)
